# revision 25
# baseline (speedup 1.0000x reference)
"""Trainium2 Bass kernel for the MoE-routing module.

Computation (B=32768, D=1024, H=512, F=100, E=16, K=2):
    h   = relu(x @ W_shared + b_shared)                  [B, H]
    a   = relu(einsum('bh,ehf', h, W1) + b1)             [B, E, F]
    o   = einsum('bef,efo', a, W2) + b2                  [B, E, 1]
    out = mean over the K routed experts of o[b, send_to[idx[b]]]

Strategy: host sorts tokens by head id and shards the sorted batch over
the 8 cores (4096 tokens each), 9 device chunks per core (two 256-token
head chunks let M1 start while the DMA backlog clears).  A head group
covers ~4 chunks, so most chunk positions hold a single head id on every
core: those need exactly the 2 routed experts, and the top-2 mean
collapses to a constant 0.5/0.5 blend folded into W2 — the select stage
merges into M3 as a 1-column matmul (no mask, no vector work).
Positions where any core crosses a head boundary run a general masked
path with EC slots (3 normally).  Per-position structure (slot count +
masked?) is uniform across cores, so one SPMD program serves all 8;
programs are cached per structure key.

All matmuls run in fp16: same 1 cycle/row PE rate as fp32r at 512-wide
moving tiles, but half the HBM traffic; final rel err ~6e-4 (fp8 was
measured at 4e-2 — over the 2e-2 budget — and is not used).

Stages (features on SBUF partitions throughout):
  M1: hT[h, t]  = relu(W_shared.T @ xT + b)       8 k-tiles as 2 halves
  M2: aT[f', t] = relu(W1sel.T @ hT + b1)         f' = slot*128 + f
  M3 single: out[t] = 0.5*(W2cat).T @ aT + b2m    1-col lhsT, merged sel
  M3 mixed:  c[j, t] = W2bd.T @ aT; out = ones.T @ (c * mask) + b2m[t]

Schedule notes (each worth measured microseconds on the NTFF trace):
- dummy PE matmuls right after the preamble keep the HAM clock-gate
  ramping, so real M1 runs at 2.4 GHz from the start;
- per-chunk x/w1 DMAs are issued from inside the compute loop two chunks
  ahead — priority-pinning them all up front parks their semaphore-
  window waits in front of the activations on the Act queue and
  serializes the whole head;
- mixed (masked) chunks are processed late: their PE->DVE->PSUM chain
  stalls the PE ~10us if placed in the DMA-limited head;
- x/wsh move as half-chunk DMAs (4 KB per-partition runs);
- out DMAs ride the gpsimd SWDGE except the last (sync ring), keeping
  the slow SWDGE end-drain off the tail critical path.
"""

import numpy as np

import concourse.mybir as mybir
from concourse import bacc
from concourse.bass_utils import run_bass_kernel_spmd
from concourse.tile import TileContext

B, D, H, F, E, TOPK = 32768, 1024, 512, 100, 16, 2
N_CORES = 8
BL = B // N_CORES          # tokens per core
CHUNK = 512                # max tokens per device-side tile loop
# smaller head chunks let M1 start while the DMA backlog clears
SIZES = (256, 256, 512, 512, 512, 512, 512, 512, 512)
OFFS = [0]
for _s in SIZES:
    OFFS.append(OFFS[-1] + _s)
assert OFFS[-1] == BL
NCH = len(SIZES)           # chunks per core
KD = D // 128              # M1 contraction tiles
NPAIR = KD // 2            # M1 contraction tile pairs (DMA granularity)
MH = H // 128              # M1 output tiles
KH = H // 128              # M2 contraction tiles

COMPUTE_DT = "float16"
CDT = mybir.dt.float16
NP_CDT = np.float16
_FP32 = mybir.dt.float32
_cache = {}


def _build_nc(key):
    """Build the SPMD program for per-position (slot count, masked) key."""
    ecs, mixed = key
    n_mixed = sum(mixed)
    max_ec = max(ecs)
    MROWS = 33                           # mask rows: slots + b2mean at row 32
    W2COLS = sum(e * e if mx else e for e, mx in zip(ecs, mixed))
    NB = MH + sum(ecs) + sum(0 if mx else 1 for mx in mixed)

    nc = bacc.Bacc("TRN2", target_bir_lowering=False, num_devices=N_CORES)

    xT_d = nc.declare_dram_parameter("xT", [D * BL], CDT, isOutput=False)
    wsh_d = nc.declare_dram_parameter("wsh", [D * H], CDT, isOutput=False)
    w1sz = [KH * 128 * e * 128 for e in ecs]
    w1off = np.cumsum([0] + w1sz).tolist()
    w1c_d = nc.declare_dram_parameter("w1c", [w1off[-1]], CDT, isOutput=False)
    w2_d = nc.declare_dram_parameter("w2", [128, W2COLS], CDT, isOutput=False)
    bias_d = nc.declare_dram_parameter("biases", [128, NB], _FP32, isOutput=False)
    bias2_d = nc.declare_dram_parameter("bias2", [1, BL], _FP32, isOutput=False)
    if n_mixed:
        mask_d = nc.declare_dram_parameter(
            "mask", [MROWS, n_mixed * CHUNK], _FP32, isOutput=False
        )
    out_d = nc.declare_dram_parameter("out", [BL], _FP32, isOutput=True)

    relu = mybir.ActivationFunctionType.Relu

    with TileContext(nc) as tc:
        with (
            tc.tile_pool(name="weights", bufs=1) as wpool,
            tc.tile_pool(name="xin", bufs=24) as xpool,
            tc.tile_pool(name="w1p", bufs=3) as w1pool,
            tc.tile_pool(name="hmid", bufs=3) as hpool,
            tc.tile_pool(name="amid", bufs=3) as apool,
            tc.tile_pool(name="small", bufs=10) as spool,
            tc.tile_pool(name="ps_h", bufs=4, space="PSUM") as ps_h,
            tc.tile_pool(name="ps_a", bufs=2, space="PSUM") as ps_a,
            tc.tile_pool(name="ps_c", bufs=1, space="PSUM") as ps_c,
            tc.tile_pool(name="ps_o", bufs=1, space="PSUM") as ps_o,
        ):
            # ---- DMAs with explicit priorities pinning queue order ----
            _prio = [0]

            def pdma(q, dst, src):
                inst = q.dma_start(dst, src)
                inst.ins.bass_priority = _prio[0]
                _prio[0] += 1
                return inst

            def xhalf_view(j, h):
                sz = SIZES[j]
                o = (OFFS[j] * D) + h * (128 * 4 * sz)
                return xT_d[o : o + 128 * 4 * sz].rearrange(
                    "(p q t) -> p q t", p=128, q=4
                )

            def wsh_view(h):
                o = h * (128 * 4 * H)
                return wsh_d[o : o + 128 * 4 * H].rearrange(
                    "(p q h) -> p q h", p=128, q=4
                )

            def w1_view(j):
                return w1c_d[w1off[j] : w1off[j + 1]].rearrange(
                    "(p k c) -> p k c", p=128, k=KH
                )

            # Chunk processing order: single-head positions first, mixed
            # (masked) positions last.  The mixed path's cross-engine chain
            # (PE -> vector mask ops -> psum reuse -> PE) stalls the PE for
            # ~10us when it sits in the DMA-limited head; at the end it
            # overlaps the drain instead.
            singles = [j for j in range(NCH) if not mixed[j]]
            mixed_js = [j for j in range(NCH) if mixed[j]]
            # mixed positions run late (their cross-engine mask chain would
            # stall the DMA-limited head) but not last: a single-head chunk
            # closes the program so the final vector chain overlaps compute.
            if len(singles) >= 2:
                order = singles[:-1] + mixed_js + singles[-1:]
            else:
                order = singles + mixed_js

            # head: wsh halves + first-processed chunk's x halves, split
            # across both HW rings (4KB per-partition runs, few issues)
            wshh = [wpool.tile([128, 4, H], CDT, name=f"wshh{h}") for h in range(2)]
            xts = [
                [
                    xpool.tile([128, 4, SIZES[j]], CDT, tag="xt", name=f"xt{j}_{h}")
                    for h in range(2)
                ]
                for j in range(NCH)
            ]
            j0 = order[0]
            pdma(nc.sync, wshh[0][:], wsh_view(0))
            pdma(nc.scalar, xts[j0][0][:], xhalf_view(j0, 0))
            pdma(nc.sync, xts[j0][1][:], xhalf_view(j0, 1))
            pdma(nc.scalar, wshh[1][:], wsh_view(1))

            # small tensors ride the gpsimd software DGE, off the hot rings
            bias_sb = wpool.tile([128, NB], _FP32)
            pdma(nc.gpsimd, bias_sb[:], bias_d[:])
            bias2_sb = wpool.tile([1, BL], _FP32)
            pdma(nc.gpsimd, bias2_sb[:], bias2_d[:])
            w2_sb = wpool.tile([128, W2COLS], CDT)
            pdma(nc.gpsimd, w2_sb[:], w2_d[:])
            if n_mixed:
                mask_sb = wpool.tile([MROWS, n_mixed * CHUNK], _FP32)
                pdma(nc.gpsimd, mask_sb[:], mask_d[:])
                ones_sb = wpool.tile([max_ec, 1], CDT)
                nc.vector.memset(ones_sb[:], 1.0)

            # head prefetch: w1 + x for the first two processed chunks,
            # priority-pinned.  Later chunks' DMAs are issued from inside
            # the compute loop (2 chunks ahead) so their semaphore-window
            # waits never sit in front of activations on the Act queue.
            w1sb = [None] * NCH
            w1sb[j0] = w1pool.tile(
                [128, KH, ecs[j0] * 128], CDT, tag="w1", name=f"w1_{j0}"
            )
            pdma(nc.sync, w1sb[j0][:], w1_view(j0))
            j1 = order[1]
            pdma(nc.sync, xts[j1][0][:], xhalf_view(j1, 0))
            pdma(nc.scalar, xts[j1][1][:], xhalf_view(j1, 1))
            w1sb[j1] = w1pool.tile(
                [128, KH, ecs[j1] * 128], CDT, tag="w1", name=f"w1_{j1}"
            )
            pdma(nc.scalar, w1sb[j1][:], w1_view(j1))

            def fetch_chunk(j, oi):
                nc.sync.dma_start(xts[j][0][:], xhalf_view(j, 0))
                nc.scalar.dma_start(xts[j][1][:], xhalf_view(j, 1))
                w1sb[j] = w1pool.tile(
                    [128, KH, ecs[j] * 128], CDT, tag="w1", name=f"w1_{j}"
                )
                (nc.scalar if oi % 2 else nc.sync).dma_start(
                    w1sb[j][:], w1_view(j)
                )

            # ---- PE warm-up: back-to-back dummy matmuls keep the HAM
            # clock-gate ramping to full speed while the first DMAs land,
            # so real M1 work runs at 2.4 GHz from the start.  Results go
            # to a scratch psum tile that nothing reads.
            warm_sb = wpool.tile([128, 512], CDT)
            nc.vector.memset(warm_sb[:], 0.0)
            warm_ps = ps_a.tile([128, 512], _FP32, tag="ps_a", name="warm_ps")
            for wi in range(7):
                nc.tensor.matmul(
                    warm_ps[:],
                    lhsT=warm_sb[:, :128],
                    rhs=warm_sb[:],
                    start=True,
                    stop=True,
                )

            # ---- compute ----
            # per-position col offsets into w2_sb / bias_sb / mask (host
            # packs in position order)
            w2cs, bcols, mjs = [], [], []
            _w2c, _bcol, _mj = 0, MH, 0
            for j in range(NCH):
                w2cs.append(_w2c)
                bcols.append(_bcol)
                mjs.append(_mj)
                if mixed[j]:
                    _w2c += ecs[j] * ecs[j]
                    _bcol += ecs[j]
                    _mj += 1
                else:
                    _w2c += ecs[j]
                    _bcol += ecs[j] + 1

            for oi in range(NCH):
                j = order[oi]
                ec = ecs[j]
                sz = SIZES[j]
                xt = xts[j]
                w2c = w2cs[j]
                bcol = bcols[j]
                mj = mjs[j]
                if oi + 2 < NCH:
                    fetch_chunk(order[oi + 2], oi)

                # M1: hT = relu(W_shared.T @ xT + b)
                hT = hpool.tile([128, MH, sz], CDT, tag="hT", name=f"hT{j}")
                if oi <= 1:
                    # k-outer so matmuls start as the first pair DMAs land
                    phs = [
                        ps_h.tile([128, sz], _FP32, tag="ps_h", name=f"ph{j}_{m}")
                        for m in range(MH)
                    ]
                    for p in range(NPAIR):
                        for i in range(2):
                            h, q = p // 2, (p % 2) * 2 + i
                            for m in range(MH):
                                nc.tensor.matmul(
                                    phs[m][:],
                                    lhsT=wshh[h][:, q, m * 128 : (m + 1) * 128],
                                    rhs=xt[h][:, q, :],
                                    start=(p == 0 and i == 0),
                                    stop=(p == NPAIR - 1 and i == 1),
                                )
                    for m in range(MH):
                        nc.scalar.activation(
                            hT[:, m, :], phs[m][:], relu, bias=bias_sb[:, m : m + 1]
                        )
                else:
                    for m in range(MH):
                        ph = ps_h.tile(
                            [128, sz], _FP32, tag="ps_h", name=f"ph{j}_{m}"
                        )
                        for p in range(NPAIR):
                            for i in range(2):
                                h, q = p // 2, (p % 2) * 2 + i
                                nc.tensor.matmul(
                                    ph[:],
                                    lhsT=wshh[h][:, q, m * 128 : (m + 1) * 128],
                                    rhs=xt[h][:, q, :],
                                    start=(p == 0 and i == 0),
                                    stop=(p == NPAIR - 1 and i == 1),
                                )
                        nc.scalar.activation(
                            hT[:, m, :], ph[:], relu, bias=bias_sb[:, m : m + 1]
                        )

                # M2: aT = relu(W1sel.T @ hT + b1)
                aT = apool.tile([128, ec, sz], CDT, tag="aT", name=f"aT{j}")
                for mi in range(ec):
                    pa = ps_a.tile([128, sz], _FP32, tag="ps_a", name=f"pa{j}_{mi}")
                    for k in range(KH):
                        nc.tensor.matmul(
                            pa[:],
                            lhsT=w1sb[j][:, k, mi * 128 : (mi + 1) * 128],
                            rhs=hT[:, k, :],
                            start=(k == 0),
                            stop=(k == KH - 1),
                        )
                    nc.scalar.activation(
                        aT[:, mi, :], pa[:], relu,
                        bias=bias_sb[:, bcol + mi : bcol + mi + 1],
                    )

                # M3 + select
                t0 = OFFS[j]
                ot = spool.tile([1, sz], _FP32, tag="ot", name=f"ot{j}")
                if not mixed[j]:
                    po = ps_o.tile([1, sz], _FP32, tag="ps_o", name=f"po{j}")
                    for k in range(ec):
                        nc.tensor.matmul(
                            po[:],
                            lhsT=w2_sb[:, w2c + k : w2c + k + 1],
                            rhs=aT[:, k, :],
                            start=(k == 0),
                            stop=(k == ec - 1),
                        )
                    # per-token b2-mean row add on the (idle) vector engine:
                    # keeps the Act queue off the chunk's output chain
                    nc.vector.tensor_add(
                        ot[:], po[:], bias2_sb[0:1, t0 : t0 + sz]
                    )
                else:
                    pc = ps_c.tile([ec, sz], _FP32, tag="ps_c", name=f"pc{j}")
                    for k in range(ec):
                        nc.tensor.matmul(
                            pc[:],
                            lhsT=w2_sb[:, w2c + k * ec : w2c + (k + 1) * ec],
                            rhs=aT[:, k, :],
                            start=(k == 0),
                            stop=(k == ec - 1),
                        )
                    msel = spool.tile([ec, sz], CDT, tag="msel", name=f"msel{j}")
                    nc.vector.tensor_mul(
                        msel[:], pc[:],
                        mask_sb[:ec, mj * CHUNK : mj * CHUNK + sz],
                    )
                    po = ps_o.tile([1, sz], _FP32, tag="ps_o", name=f"pom{j}")
                    nc.tensor.matmul(
                        po[:], lhsT=ones_sb[:ec, :], rhs=msel[:], start=True, stop=True
                    )
                    nc.vector.tensor_add(
                        ot[:], po[:],
                        mask_sb[32:33, mj * CHUNK : mj * CHUNK + sz],
                    )
                # out DMAs ride the gpsimd SWDGE (a HW-ring out DMA would
                # block later input DMA issues behind its wait) — except the
                # final one, which uses the now-idle sync ring so gpsimd's
                # slow SWDGE end-drain moves off the tail critical path.
                outq = nc.sync if oi == NCH - 1 else nc.gpsimd
                outq.dma_start(
                    out_d[t0 : t0 + sz].rearrange("(o t) -> o t", o=1), ot[:]
                )

    nc.compile()
    return nc


def get_nc(key):
    if key not in _cache:
        _cache[key] = _build_nc(key)
    return _cache[key]


def prepare(inputs):
    """Host-side routing/sorting/sharding. Returns (key, in_maps, perm)."""
    x = np.asarray(inputs["x"], dtype=np.float32)
    idx = np.asarray(inputs["idx"]).astype(np.int64).reshape(B)
    W_shared = np.asarray(inputs["W_shared"], dtype=np.float32)
    b_shared = np.asarray(inputs["b_shared"], dtype=np.float32).reshape(H)
    W1 = np.asarray(inputs["W1"], dtype=np.float32)
    b1 = np.asarray(inputs["b1"], dtype=np.float32).reshape(E, F)
    W2 = np.asarray(inputs["W2"], dtype=np.float32).reshape(E, F)
    b2 = np.asarray(inputs["b2"], dtype=np.float32).reshape(E)
    send_to = np.asarray(inputs["send_to"]).astype(np.int64)

    perm = np.argsort(idx, kind="stable")
    idx_s = idx[perm]
    x_s = x[perm]
    routes_s = send_to[idx_s]                      # [B, K] sorted routes

    # per-position structure: slot count + masked?, uniform across cores
    slot_lists = [[None] * NCH for _ in range(N_CORES)]
    ecs, mixed = [], []
    for j in range(NCH):
        ec_j, mx_j = 2, False
        for c in range(N_CORES):
            sl = slice(c * BL + OFFS[j], c * BL + OFFS[j + 1])
            experts = np.unique(routes_s[sl])
            slot_lists[c][j] = experts
            ec_j = max(ec_j, len(experts))
            if len(np.unique(idx_s[sl])) > 1:
                mx_j = True
        ecs.append(ec_j)
        mixed.append(mx_j)
    ecs, mixed = tuple(ecs), tuple(mixed)
    n_mixed = sum(mixed)
    max_ec = max(ecs)
    MROWS = 33

    # wsh half blocks [half, 128, 4, H]
    wshr = W_shared.reshape(2, 4, 128, H)
    wsh_flat = np.ascontiguousarray(wshr.transpose(0, 2, 1, 3)).astype(NP_CDT).ravel()

    in_maps = []
    for c in range(N_CORES):
        xc = x_s[c * BL : (c + 1) * BL]
        # per-chunk half blocks [half, 128, 4, sz]
        xparts = []
        for j in range(NCH):
            xj = xc[OFFS[j] : OFFS[j + 1]].reshape(SIZES[j], 2, 4, 128)
            xparts.append(
                np.ascontiguousarray(xj.transpose(1, 3, 2, 0)).astype(NP_CDT).ravel()
            )
        xT = np.concatenate(xparts)

        w1_parts = []
        w2_cols = []
        bias_cols = [b_shared.reshape(MH, 128).T]
        bias2_row = np.zeros((1, BL), np.float32)
        mask_cols = np.zeros((MROWS, max(n_mixed, 1) * CHUNK), np.float32)
        mj = 0
        for j in range(NCH):
            sl = slice(c * BL + OFFS[j], c * BL + OFFS[j + 1])
            ec = ecs[j]
            slots = np.full(ec, -1, dtype=np.int64)
            el = slot_lists[c][j]
            slots[: len(el)] = el

            w1sel = np.zeros((H, ec * 128), np.float32)
            b1sel = np.zeros(ec * 128, np.float32)
            for mi, e in enumerate(slots):
                if e < 0:
                    continue
                w1sel[:, mi * 128 : mi * 128 + F] = W1[e]
                b1sel[mi * 128 : mi * 128 + F] = b1[e]
            w1_parts.append(
                np.ascontiguousarray(
                    w1sel.reshape(KH, 128, ec * 128).transpose(1, 0, 2)
                ).astype(NP_CDT).ravel()
            )
            bias_cols.append(b1sel.reshape(ec, 128).T)

            r = routes_s[sl]                        # [CHUNK, K]
            if not mixed[j]:
                w2m = np.zeros((128, ec), np.float32)
                for e in r[0]:  # routes with multiplicity
                    mi = int(np.where(slots == e)[0][0])
                    w2m[:F, mi] += W2[e] / r.shape[1]
                w2_cols.append(w2m)
                col = np.zeros((128, 1), np.float32)
                col[0, 0] = b2[r[0]].mean()
                bias_cols.append(col)
                bias2_row[0, OFFS[j] : OFFS[j + 1]] = b2[r[0]].mean()
            else:
                w2full = np.zeros((ec * 128, ec), np.float32)
                for mi, e in enumerate(slots):
                    if e < 0:
                        continue
                    w2full[mi * 128 : mi * 128 + F, mi] = W2[e]
                w2_cols.append(
                    w2full.reshape(ec, 128, ec).transpose(1, 0, 2).reshape(128, ec * ec)
                )
                sz = SIZES[j]
                for k in range(r.shape[1]):
                    hit = slots[:, None] == r[None, :, k]
                    mask_cols[:ec, mj * CHUNK : mj * CHUNK + sz] += (
                        hit.astype(np.float32) / r.shape[1]
                    )
                mask_cols[32, mj * CHUNK : mj * CHUNK + sz] = b2[r].mean(axis=1)
                mj += 1

        in_map = {
            "xT": xT,
            "wsh": wsh_flat,
            "w1c": np.concatenate(w1_parts),
            "w2": np.concatenate(w2_cols, axis=1).astype(NP_CDT),
            "biases": np.ascontiguousarray(
                np.concatenate(bias_cols, axis=1)
            ).astype(np.float32),
            "bias2": bias2_row,
        }
        if n_mixed:
            in_map["mask"] = mask_cols
        in_maps.append(in_map)
    return (ecs, mixed), in_maps, perm


def kernel(**inputs) -> np.ndarray:
    key, in_maps, perm = prepare(inputs)
    nc = get_nc(key)
    res = run_bass_kernel_spmd(nc, in_maps, list(range(N_CORES)))
    out_sorted = np.concatenate([res.results[c]["out"] for c in range(N_CORES)])
    out = np.empty(B, dtype=np.float32)
    out[perm] = out_sorted
    return out.reshape(B, 1)
